# revision 20
# baseline (speedup 1.0000x reference)
"""Chamfer loss kernel v2 for Trainium2, 8 NeuronCores.

Math: T[i,j] = -||g_i - p_j||^2 / 2 computed directly in PSUM by ONE
K=13 bf16 matmul per tile using an exact hi/lo split:
  g.p  = gh.ph + gh.pl + gl.ph   (+ gl.pl ~ 1e-5, dropped)
  plus rows pairing ones with hi/lo splits of -|p|^2/2 and -|g|^2/2.
Since the PE reduces along the partition (K) dim for free, K=13 bf16
costs the same as K=4, and bf16 streams 1 col/cycle vs fp32's 4.

One T surface per batch serves BOTH chamfer directions:
  loss2 part: row max (over j)  -> DVE tensor_scalar accum-max (bf16 4x,
              fp8 junk main-out to halve the write traffic)
  loss1 part: col max (over i)  -> DVE tensor_tensor max accumulation
              across row tiles (bf16 2x), partition-axis tail via
              gpsimd partition_all_reduce(max) + DVE accum-add.
ACT stages each PSUM group to SBUF bf16 (safe: T near its max is tiny,
so bf16 rounding there is ~1e-4 absolute).

loss = -2 * (sum_i max_j T + sum_j max_i T) / (B*N), summed on host.

Sharding: batch dim 16 -> 2 per core. All input prep (hi/lo splits,
norm rows, transpose to [13, n] blocks) is host-side numpy.
"""

import sys

import numpy as np

sys.path.insert(0, "/opt/trn_rl_repo")

import ml_dtypes  # noqa: E402

import concourse.bass as bass  # noqa: E402
import concourse.mybir as mybir  # noqa: E402
import concourse.tile as tile  # noqa: E402
from concourse import bacc  # noqa: E402
from concourse.bass_utils import run_bass_kernel_spmd  # noqa: E402

BF16 = ml_dtypes.bfloat16

B, N_FULL, D = 16, 4096, 3
NCORES = 8
BLOC = B // NCORES  # batches per core
KR = 13  # matmul contraction rows (padded to KPAD on device)
KPAD = 16  # padded stationary partition count (alignment)
FREE = 512  # one PSUM bank of fp32
GROUP = 2048  # psum tile free size (4 banks)

_built = {}


def build(n=N_FULL, bloc=BLOC, reps=1, gcols=0, tail="r", junk8=True,
          probe="full", mmw=FREE, ttc=0, wide=False, tsp=1):
    """Per-core Bass module. Returns (nc, input names..., output name).

    gcols: of each 2048-col psum group, how many columns of the col-max
      accumulation run on GPSIMD (rest on DVE). DVE pays a pipeline-drain
      ~= op duration on every op, so offloading to the otherwise-idle
      GPSIMD balances the engines.
    tail: "r" = gpsimd partition_all_reduce for the col-path partition
      reduction; "t" = PE transpose + DVE reduce_max fallback.
    """
    key = (n, bloc, reps, gcols, tail, junk8, probe, mmw, ttc, wide, tsp)
    if key in _built:
        return _built[key]
    do_act = probe != "mm"
    do_row = probe in ("full", "row")
    do_col = probe in ("full", "col")

    group = min(GROUP, n)
    nt = n // 128  # row tiles
    ngrp = n // group  # psum groups per row strip
    mmw = min(mmw, group)
    nch = group // mmw  # matmuls per psum group
    ncol = group // 128  # transpose chunks per psum group
    ngrp_tt = max(0, ngrp - tsp)  # pred groups col-maxed via tt
    jt0 = (ngrp_tt * group) // 128  # first pred tile handled via T'
    ttc = ttc or group  # col-max tt chunk size
    ntc = group // ttc

    gcols = min(gcols, group)
    dcols = group - gcols  # DVE's share of each col-max group

    nc = bacc.Bacc("TRN2", target_bir_lowering=False, debug=False)
    from concourse import bass_isa
    bf = mybir.dt.bfloat16
    fp = mybir.dt.float32
    jdt = mybir.dt.float8e4 if junk8 else bf

    L_d = nc.dram_tensor("Lblk", [bloc, KR, n], bf, kind="ExternalInput")
    R_d = nc.dram_tensor("Rblk", [bloc, KR, n], bf, kind="ExternalInput")
    I_d = nc.dram_tensor("ident", [128, 128], bf, kind="ExternalInput")
    out_d = nc.dram_tensor("out", [128, 3 * bloc], fp, kind="ExternalOutput")

    with tile.TileContext(nc) as tc:
        with (
            tc.tile_pool(name="blocks", bufs=1) as blocks,
            tc.tile_pool(name="stg", bufs=4) as stg,
            tc.tile_pool(name="scr", bufs=2) as scr,
            tc.tile_pool(name="colp", bufs=2) as colp,
            tc.tile_pool(name="rowp", bufs=2) as rowp,
            tc.tile_pool(name="small", bufs=1) as small,
            tc.tile_pool(name="psum", bufs=2, space="PSUM") as psum_pool,
        ):
            out_sb = small.tile([128, 3 * bloc], fp, tag="out_sb")
            nc.scalar.memzero(out_sb[:])
            ident = small.tile([128, 128], bf, tag="ident")
            nc.sync.dma_start(out=ident[:], in_=I_d[:, :])

            lbs, rbs = [], []
            for b in range(bloc):
                lb = blocks.tile([KR, n], bf, tag=f"lb{b}")
                rb = blocks.tile([KR, n], bf, tag=f"rb{b}")
                nc.sync.dma_start(out=lb[:], in_=L_d[b])
                nc.sync.dma_start(out=rb[:], in_=R_d[b])
                lbs.append(lb)
                rbs.append(rb)

            def emit_main():
                for b in range(bloc):
                    lb, rb = lbs[b], rbs[b]
                    cm_n = max(ngrp_tt * group, 128)
                    colmax = colp.tile([128, cm_n], bf, tag="colmax")
                    nrp = nt if wide else nt * ngrp
                    rowpart = rowp.tile([128, nrp], fp, tag="rowpart")
                    colpart = rowp.tile([128, ngrp * ncol], fp, tag="colpart")
                    for t in range(nt):
                        w = lb[:, t * 128:(t + 1) * 128]
                        st4 = None
                        if wide and do_act:
                            st4 = stg.tile([128, n], bf, tag="st")
                        for g in range(ngrp):
                            ps = psum_pool.tile([128, group], fp, tag="ps")
                            for c in range(nch):
                                j0 = g * group + c * mmw
                                nc.tensor.matmul(
                                    ps[:, c * mmw:(c + 1) * mmw],
                                    w,
                                    rb[:, j0:j0 + mmw],
                                )
                            if not do_act:
                                continue
                            if wide:
                                nc.scalar.copy(
                                    st4[:, g * group:(g + 1) * group], ps[:]
                                )
                                continue
                            st = stg.tile([128, group], bf, tag="st")
                            nc.scalar.copy(st[:], ps[:])
                            gi = t * ngrp + g
                            col_g = do_col and g < ngrp_tt
                            csl = (colmax[:, g * group:(g + 1) * group]
                                   if col_g else None)
                            if t == 0 and col_g and do_row:
                                # fused: row accum-max + colmax init copy
                                nc.vector.tensor_scalar(
                                    out=csl, in0=st[:], scalar1=1.0,
                                    scalar2=None, op0=mybir.AluOpType.mult,
                                    op1=mybir.AluOpType.max,
                                    accum_out=rowpart[:, gi:gi + 1],
                                )
                                continue
                            if do_row:
                                sc = scr.tile([128, group], jdt, tag="sc")
                                nc.vector.tensor_scalar(
                                    out=sc[:], in0=st[:], scalar1=1.0,
                                    scalar2=None, op0=mybir.AluOpType.mult,
                                    op1=mybir.AluOpType.max,
                                    accum_out=rowpart[:, gi:gi + 1],
                                )
                            if col_g:
                                if t == 0:
                                    nc.vector.tensor_scalar_mul(
                                        csl, st[:], 1.0
                                    )
                                else:
                                    for u in range(ntc):
                                        sl = slice(u * ttc, (u + 1) * ttc)
                                        nc.vector.tensor_tensor(
                                            out=csl[:, sl], in0=st[:, sl],
                                            in1=csl[:, sl],
                                            op=mybir.AluOpType.max,
                                        )
                        if not (wide and do_act):
                            continue
                        # wide: one row ts + one col tt per tile
                        if t == 0 and do_col and do_row:
                            nc.vector.tensor_scalar(
                                out=colmax[:], in0=st4[:], scalar1=1.0,
                                scalar2=None, op0=mybir.AluOpType.mult,
                                op1=mybir.AluOpType.max,
                                accum_out=rowpart[:, t:t + 1],
                            )
                            continue
                        if do_row:
                            sc = scr.tile([128, n], jdt, tag="sc")
                            nc.vector.tensor_scalar(
                                out=sc[:], in0=st4[:], scalar1=1.0,
                                scalar2=None, op0=mybir.AluOpType.mult,
                                op1=mybir.AluOpType.max,
                                accum_out=rowpart[:, t:t + 1],
                            )
                        if do_col:
                            if t == 0:
                                nc.vector.tensor_scalar_mul(
                                    colmax[:], st4[:], 1.0
                                )
                            else:
                                nc.vector.tensor_tensor(
                                    out=colmax[:], in0=st4[:], in1=colmax[:],
                                    op=mybir.AluOpType.max,
                                )
                    # row tail: max over groups, sum over tiles
                    if do_row:
                        rmax = rowp.tile([128, nt], fp, tag="rmax")
                        if wide:
                            rmax = rowpart
                        elif ngrp > 1:
                            nc.vector.reduce_max(
                                rmax[:],
                                rowpart[:].rearrange(
                                    "p (t g) -> p t g", g=ngrp
                                ),
                                axis=mybir.AxisListType.X,
                            )
                        else:
                            nc.vector.tensor_scalar_mul(
                                rmax[:], rowpart[:], 1.0
                            )
                        nc.vector.reduce_sum(
                            out_sb[:, 2 * b:2 * b + 1], rmax[:],
                            axis=mybir.AxisListType.X,
                        )
                    # col tail: reduce colmax over the partition axis
                    if not do_col or ngrp_tt == 0:
                        pass
                    elif tail == "r":
                        nw = ngrp_tt * group
                        car = colp.tile([128, nw], bf, tag="car")
                        nc.gpsimd.partition_all_reduce(
                            car[:], colmax[:, 0:nw], channels=128,
                            reduce_op=bass_isa.ReduceOp.max,
                        )
                        cjunk = rowp.tile([1, nw], jdt, tag="cjunk")
                        nc.vector.tensor_scalar(
                            out=cjunk[:], in0=car[0:1, :], scalar1=1.0,
                            scalar2=None, op0=mybir.AluOpType.mult,
                            op1=mybir.AluOpType.add,
                            accum_out=out_sb[0:1, 2 * b + 1:2 * b + 2],
                        )
                    if do_col and tsp > 0:
                        t2part = rowp.tile([128, (nt - jt0) * ngrp], fp,
                                           tag="t2part")
                        for jt in range(jt0, nt):
                            w2 = rb[:, jt * 128:(jt + 1) * 128]
                            for g2 in range(ngrp):
                                ps2 = psum_pool.tile([128, group], fp,
                                                     tag="ps")
                                for c2 in range(nch):
                                    j2 = g2 * group + c2 * mmw
                                    nc.tensor.matmul(
                                        ps2[:, c2 * mmw:(c2 + 1) * mmw],
                                        w2,
                                        lb[:, j2:j2 + mmw],
                                    )
                                st2 = stg.tile([128, group], bf, tag="st")
                                nc.scalar.copy(st2[:], ps2[:])
                                g2i = (jt - jt0) * ngrp + g2
                                sc2 = scr.tile([128, group], jdt, tag="sc")
                                nc.vector.tensor_scalar(
                                    out=sc2[:], in0=st2[:], scalar1=1.0,
                                    scalar2=None, op0=mybir.AluOpType.mult,
                                    op1=mybir.AluOpType.max,
                                    accum_out=t2part[:, g2i:g2i + 1],
                                )
                        t2max = rowp.tile([128, nt - jt0], fp, tag="t2max")
                        if ngrp > 1:
                            nc.vector.reduce_max(
                                t2max[:],
                                t2part[:].rearrange(
                                    "p (t g) -> p t g", g=ngrp
                                ),
                                axis=mybir.AxisListType.X,
                            )
                        else:
                            nc.vector.tensor_scalar_mul(
                                t2max[:], t2part[:], 1.0
                            )
                        nc.vector.reduce_sum(
                            out_sb[:, 2 * bloc + b:2 * bloc + b + 1],
                            t2max[:],
                            axis=mybir.AxisListType.X,
                        )
                    elif False:
                        for h in range(ngrp):
                            pst = psum_pool.tile([128, group], fp, tag="ps")
                            pstb = pst[:].bitcast(bf)  # [128, 2*group] bf16
                            for cc in range(ncol):
                                j0 = h * group + cc * 128
                                nc.tensor.transpose(
                                    pstb[:, cc * 128:(cc + 1) * 128],
                                    colmax[:, j0:j0 + 128],
                                    ident[:],
                                )
                            nc.vector.reduce_max(
                                colpart[:, h * ncol:(h + 1) * ncol],
                                pstb[:, 0:ncol * 128].rearrange(
                                    "p (c f) -> p c f", f=128
                                ),
                                axis=mybir.AxisListType.X,
                            )
                        nc.vector.reduce_sum(
                            out_sb[:, 2 * b + 1:2 * b + 2], colpart[:],
                            axis=mybir.AxisListType.X,
                        )

            body = emit_main
            if reps == 1:
                body()
            else:
                # unroll 4 bodies per For_i iteration when possible: each
                # For_i iteration carries an all-engine barrier, so fewer
                # iterations = less drain/refill per rep
                U = 4 if reps % 4 == 0 else 1
                with tc.For_i(0, reps // U, 1):
                    for _ in range(U):
                        body()

            nc.sync.dma_start(out=out_d[:], in_=out_sb[:])

    nc.compile()
    _built[key] = (nc, "Lblk", "Rblk", "ident", "out")
    return _built[key]


def _split(x):
    """fp32 array -> (hi bf16, lo bf16) with hi+lo == x to ~1e-5 rel."""
    hi = x.astype(BF16)
    lo = (x - hi.astype(np.float32)).astype(BF16)
    return hi, lo


def make_blocks(g, p):
    """g, p: [n, 3] fp32 -> (L, R) [13, n] bf16 matmul blocks.

    out[m, j] = sum_k L[k, m] * R[k, j]
              = gh.ph + gh.pl + gl.ph + (-|p|^2/2) + (-|g|^2/2)
              ~ g.p - |p|^2/2 - |g|^2/2 = -||g - p||^2 / 2
    """
    n = g.shape[0]
    gh, gl = _split(g)
    ph, pl = _split(p)
    gn = (-0.5 * np.square(g.astype(np.float64)).sum(-1)).astype(np.float32)
    pn = (-0.5 * np.square(p.astype(np.float64)).sum(-1)).astype(np.float32)
    gnh, gnl = _split(gn)
    pnh, pnl = _split(pn)
    ones = np.ones((1, n), dtype=BF16)

    L = np.empty((KR, n), dtype=BF16)
    L[0:3] = gh.T
    L[3:6] = gh.T
    L[6:9] = gl.T
    L[9] = ones
    L[10] = ones
    L[11] = gnh
    L[12] = gnl

    R = np.empty((KR, n), dtype=BF16)
    R[0:3] = ph.T
    R[3:6] = pl.T
    R[6:9] = ph.T
    R[9] = pnh
    R[10] = pnl
    R[11] = ones
    R[12] = ones
    return np.ascontiguousarray(L), np.ascontiguousarray(R)


def shard_inputs(preds, gts, bloc=BLOC, ncores=NCORES):
    preds = np.asarray(preds, dtype=np.float32)
    gts = np.asarray(gts, dtype=np.float32)
    n = preds.shape[1]
    ident = np.eye(128, dtype=BF16)
    in_maps = []
    for c in range(ncores):
        Ls = np.empty((bloc, KR, n), dtype=BF16)
        Rs = np.empty((bloc, KR, n), dtype=BF16)
        for b in range(bloc):
            Ls[b], Rs[b] = make_blocks(gts[c * bloc + b], preds[c * bloc + b])
        in_maps.append({"Lblk": Ls, "Rblk": Rs, "ident": ident})
    return in_maps


def combine_outputs(outs, n=N_FULL, b=B):
    tot = np.sum([o.astype(np.float64).sum() for o in outs])
    return np.float32(-2.0 * tot / (b * n))


def kernel(preds, gts):
    nc, _, _, _, on = build()
    in_maps = shard_inputs(preds, gts)
    res = run_bass_kernel_spmd(nc, in_maps, core_ids=list(range(NCORES)))
    return combine_outputs([r[on] for r in res.results])


def _numpy_chamfer(preds, gts):
    tot = 0.0
    for b_ in range(preds.shape[0]):
        gg = (gts[b_] ** 2).sum(-1)
        pp = (preds[b_] ** 2).sum(-1)
        zz = gts[b_] @ preds[b_].T
        P = gg[:, None] + pp[None, :] - 2 * zz
        tot += P.min(axis=0).mean() + P.min(axis=1).mean()
    return tot / preds.shape[0]


if __name__ == "__main__":
    from concourse.bass_interp import CoreSim

    n = int(sys.argv[1]) if len(sys.argv) > 1 else 512
    bloc = int(sys.argv[2]) if len(sys.argv) > 2 else 1
    reps = int(sys.argv[3]) if len(sys.argv) > 3 else 1
    nc, ln, rn, idn, on = build(n=n, bloc=bloc, reps=reps)
    rng = np.random.default_rng(0)
    preds = rng.standard_normal((bloc, n, D)).astype(np.float32)
    gts = rng.standard_normal((bloc, n, D)).astype(np.float32)

    sim = CoreSim(nc)
    for bi in range(bloc):
        Lb, Rb = make_blocks(gts[bi], preds[bi])
        sim.tensor(ln)[bi] = Lb
        sim.tensor(rn)[bi] = Rb
    sim.tensor(idn)[:] = np.eye(128, dtype=BF16)
    sim.simulate()
    got = combine_outputs([sim.tensor(on)], n=n, b=bloc)
    want = _numpy_chamfer(preds, gts)
    print("sim:", got, "numpy:", want, "rel err:", abs(got - want) / abs(want))



# revision 23
# speedup vs baseline: 1.1140x; 1.1140x over previous
"""Chamfer loss kernel v2 for Trainium2, 8 NeuronCores.

Math: T[i,j] = -||g_i - p_j||^2 / 2 computed directly in PSUM by ONE
K=13 bf16 matmul per tile using an exact hi/lo split:
  g.p  = gh.ph + gh.pl + gl.ph   (+ gl.pl ~ 1e-5, dropped)
  plus rows pairing ones with hi/lo splits of -|p|^2/2 and -|g|^2/2.
Since the PE reduces along the partition (K) dim for free, K=13 bf16
costs the same as K=4, and bf16 streams 1 col/cycle vs fp32's 4.

One T surface per batch serves BOTH chamfer directions:
  loss2 part: row max (over j)  -> DVE tensor_scalar accum-max (bf16 4x,
              fp8 junk main-out to halve the write traffic)
  loss1 part: col max (over i)  -> DVE tensor_tensor max accumulation
              across row tiles (bf16 2x), partition-axis tail via
              gpsimd partition_all_reduce(max) + DVE accum-add.
ACT stages each PSUM group to SBUF bf16 (safe: T near its max is tiny,
so bf16 rounding there is ~1e-4 absolute).

loss = -2 * (sum_i max_j T + sum_j max_i T) / (B*N), summed on host.

Sharding: batch dim 16 -> 2 per core. All input prep (hi/lo splits,
norm rows, transpose to [13, n] blocks) is host-side numpy.
"""

import sys

import numpy as np

sys.path.insert(0, "/opt/trn_rl_repo")

import ml_dtypes  # noqa: E402

import concourse.bass as bass  # noqa: E402
import concourse.mybir as mybir  # noqa: E402
import concourse.tile as tile  # noqa: E402
from concourse import bacc  # noqa: E402
from concourse.bass_utils import run_bass_kernel_spmd  # noqa: E402

BF16 = ml_dtypes.bfloat16

B, N_FULL, D = 16, 4096, 3
NCORES = 8
BLOC = B // NCORES  # batches per core
KR = 13  # matmul contraction rows (padded to KPAD on device)
KPAD = 16  # padded stationary partition count (alignment)
FREE = 512  # one PSUM bank of fp32
GROUP = 2048  # psum tile free size (4 banks)

_built = {}


def build(n=N_FULL, bloc=BLOC, reps=1, gcols=0, tail="r", junk8=True,
          probe="full", mmw=FREE, ttc=0, wide=False):
    """Per-core Bass module. Returns (nc, input names..., output name).

    gcols: of each 2048-col psum group, how many columns of the col-max
      accumulation run on GPSIMD (rest on DVE). DVE pays a pipeline-drain
      ~= op duration on every op, so offloading to the otherwise-idle
      GPSIMD balances the engines.
    tail: "r" = gpsimd partition_all_reduce for the col-path partition
      reduction; "t" = PE transpose + DVE reduce_max fallback.
    """
    key = (n, bloc, reps, gcols, tail, junk8, probe, mmw, ttc, wide)
    if key in _built:
        return _built[key]
    do_act = probe != "mm"
    do_row = probe in ("full", "row")
    do_col = probe in ("full", "col")

    group = min(GROUP, n)
    nt = n // 128  # row tiles
    ngrp = n // group  # psum groups per row strip
    mmw = min(mmw, group)
    nch = group // mmw  # matmuls per psum group
    ncol = group // 128  # transpose chunks per psum group
    ttc = ttc or group  # col-max tt chunk size
    ntc = group // ttc

    gcols = min(gcols, group)
    dcols = group - gcols  # DVE's share of each col-max group

    nc = bacc.Bacc("TRN2", target_bir_lowering=False, debug=False)
    from concourse import bass_isa
    bf = mybir.dt.bfloat16
    fp = mybir.dt.float32
    jdt = mybir.dt.float8e4 if junk8 else bf

    L_d = nc.dram_tensor("Lblk", [bloc, KR, n], bf, kind="ExternalInput")
    R_d = nc.dram_tensor("Rblk", [bloc, KR, n], bf, kind="ExternalInput")
    I_d = nc.dram_tensor("ident", [128, 128], bf, kind="ExternalInput")
    out_d = nc.dram_tensor("out", [128, 2 * bloc], fp, kind="ExternalOutput")

    with tile.TileContext(nc) as tc:
        with (
            tc.tile_pool(name="blocks", bufs=1) as blocks,
            tc.tile_pool(name="stg", bufs=6) as stg,
            tc.tile_pool(name="scr", bufs=2) as scr,
            tc.tile_pool(name="colp", bufs=3) as colp,
            tc.tile_pool(name="rowp", bufs=2) as rowp,
            tc.tile_pool(name="small", bufs=1) as small,
            tc.tile_pool(name="psum", bufs=2, space="PSUM") as psum_pool,
        ):
            out_sb = small.tile([128, 2 * bloc], fp, tag="out_sb")
            nc.scalar.memzero(out_sb[:])
            ident = small.tile([128, 128], bf, tag="ident")
            nc.sync.dma_start(out=ident[:], in_=I_d[:, :])

            lbs, rbs = [], []
            for b in range(bloc):
                lb = blocks.tile([KR, n], bf, tag=f"lb{b}")
                rb = blocks.tile([KR, n], bf, tag=f"rb{b}")
                nc.sync.dma_start(out=lb[:], in_=L_d[b])
                nc.sync.dma_start(out=rb[:], in_=R_d[b])
                lbs.append(lb)
                rbs.append(rb)

            def emit_main():
                for b in range(bloc):
                    lb, rb = lbs[b], rbs[b]
                    colmax = colp.tile([128, n], bf, tag="colmax")
                    nrp = nt if wide else nt * ngrp
                    rowpart = rowp.tile([128, nrp], fp, tag="rowpart")
                    colpart = rowp.tile([128, ngrp * ncol], fp, tag="colpart")
                    for t in range(nt):
                        w = lb[:, t * 128:(t + 1) * 128]
                        st4 = None
                        if wide and do_act:
                            st4 = stg.tile([128, n], bf, tag="st")
                        for g in range(ngrp):
                            ps = psum_pool.tile([128, group], fp, tag="ps")
                            for c in range(nch):
                                j0 = g * group + c * mmw
                                nc.tensor.matmul(
                                    ps[:, c * mmw:(c + 1) * mmw],
                                    w,
                                    rb[:, j0:j0 + mmw],
                                )
                            if not do_act:
                                continue
                            if wide:
                                nc.scalar.copy(
                                    st4[:, g * group:(g + 1) * group], ps[:]
                                )
                                continue
                            st = stg.tile([128, group], bf, tag="st")
                            nc.scalar.copy(st[:], ps[:])
                            gi = t * ngrp + g
                            csl = colmax[:, g * group:(g + 1) * group]
                            # NOTE: a fused variant (ts main-out -> csl
                            # bf16 + accum) drops the DVE to 1x mode when
                            # accum_out is present with a 16-bit main out;
                            # two separate 4x ops are ~2x faster.
                            if do_row:
                                sc = scr.tile([128, group], jdt, tag="sc")
                                nc.vector.tensor_scalar(
                                    out=sc[:], in0=st[:], scalar1=1.0,
                                    scalar2=None, op0=mybir.AluOpType.mult,
                                    op1=mybir.AluOpType.max,
                                    accum_out=rowpart[:, gi:gi + 1],
                                )
                            if do_col:
                                if t == 0:
                                    nc.vector.tensor_scalar_mul(
                                        csl, st[:], 1.0
                                    )
                                else:
                                    for u in range(ntc):
                                        sl = slice(u * ttc, (u + 1) * ttc)
                                        nc.vector.tensor_tensor(
                                            out=csl[:, sl], in0=st[:, sl],
                                            in1=csl[:, sl],
                                            op=mybir.AluOpType.max,
                                        )
                        if not (wide and do_act):
                            continue
                        # wide: one row ts + one col tt per tile
                        if t == 0 and do_col and do_row:
                            nc.vector.tensor_scalar(
                                out=colmax[:], in0=st4[:], scalar1=1.0,
                                scalar2=None, op0=mybir.AluOpType.mult,
                                op1=mybir.AluOpType.max,
                                accum_out=rowpart[:, t:t + 1],
                            )
                            continue
                        if do_row:
                            sc = scr.tile([128, n], jdt, tag="sc")
                            nc.vector.tensor_scalar(
                                out=sc[:], in0=st4[:], scalar1=1.0,
                                scalar2=None, op0=mybir.AluOpType.mult,
                                op1=mybir.AluOpType.max,
                                accum_out=rowpart[:, t:t + 1],
                            )
                        if do_col:
                            if t == 0:
                                nc.vector.tensor_scalar_mul(
                                    colmax[:], st4[:], 1.0
                                )
                            else:
                                nc.vector.tensor_tensor(
                                    out=colmax[:], in0=st4[:], in1=colmax[:],
                                    op=mybir.AluOpType.max,
                                )
                    # row tail: max over groups, sum over tiles
                    if do_row:
                        rmax = rowp.tile([128, nt], fp, tag="rmax")
                        if wide:
                            rmax = rowpart
                        elif ngrp > 1:
                            nc.vector.reduce_max(
                                rmax[:],
                                rowpart[:].rearrange(
                                    "p (t g) -> p t g", g=ngrp
                                ),
                                axis=mybir.AxisListType.X,
                            )
                        else:
                            nc.vector.tensor_scalar_mul(
                                rmax[:], rowpart[:], 1.0
                            )
                        nc.vector.reduce_sum(
                            out_sb[:, 2 * b:2 * b + 1], rmax[:],
                            axis=mybir.AxisListType.X,
                        )
                    # col tail: reduce colmax over the partition axis
                    if not do_col:
                        pass
                    elif tail == "r":
                        car = colp.tile([128, n], bf, tag="car")
                        nc.gpsimd.partition_all_reduce(
                            car[:], colmax[:], channels=128,
                            reduce_op=bass_isa.ReduceOp.max,
                        )
                        cjunk = rowp.tile([1, n], jdt, tag="cjunk")
                        nc.vector.tensor_scalar(
                            out=cjunk[:], in0=car[0:1, :], scalar1=1.0,
                            scalar2=None, op0=mybir.AluOpType.mult,
                            op1=mybir.AluOpType.add,
                            accum_out=out_sb[0:1, 2 * b + 1:2 * b + 2],
                        )
                    else:
                        for h in range(ngrp):
                            pst = psum_pool.tile([128, group], fp, tag="ps")
                            pstb = pst[:].bitcast(bf)  # [128, 2*group] bf16
                            for cc in range(ncol):
                                j0 = h * group + cc * 128
                                nc.tensor.transpose(
                                    pstb[:, cc * 128:(cc + 1) * 128],
                                    colmax[:, j0:j0 + 128],
                                    ident[:],
                                )
                            nc.vector.reduce_max(
                                colpart[:, h * ncol:(h + 1) * ncol],
                                pstb[:, 0:ncol * 128].rearrange(
                                    "p (c f) -> p c f", f=128
                                ),
                                axis=mybir.AxisListType.X,
                            )
                        nc.vector.reduce_sum(
                            out_sb[:, 2 * b + 1:2 * b + 2], colpart[:],
                            axis=mybir.AxisListType.X,
                        )

            body = emit_main
            if reps == 1:
                body()
            else:
                # unroll 4 bodies per For_i iteration when possible: each
                # For_i iteration carries an all-engine barrier, so fewer
                # iterations = less drain/refill per rep
                U = 4 if reps % 4 == 0 else 1
                with tc.For_i(0, reps // U, 1):
                    for _ in range(U):
                        body()

            nc.sync.dma_start(out=out_d[:], in_=out_sb[:])

    nc.compile()
    _built[key] = (nc, "Lblk", "Rblk", "ident", "out")
    return _built[key]


def _split(x):
    """fp32 array -> (hi bf16, lo bf16) with hi+lo == x to ~1e-5 rel."""
    hi = x.astype(BF16)
    lo = (x - hi.astype(np.float32)).astype(BF16)
    return hi, lo


def make_blocks(g, p):
    """g, p: [n, 3] fp32 -> (L, R) [13, n] bf16 matmul blocks.

    out[m, j] = sum_k L[k, m] * R[k, j]
              = gh.ph + gh.pl + gl.ph + (-|p|^2/2) + (-|g|^2/2)
              ~ g.p - |p|^2/2 - |g|^2/2 = -||g - p||^2 / 2
    """
    n = g.shape[0]
    gh, gl = _split(g)
    ph, pl = _split(p)
    gn = (-0.5 * np.square(g.astype(np.float64)).sum(-1)).astype(np.float32)
    pn = (-0.5 * np.square(p.astype(np.float64)).sum(-1)).astype(np.float32)
    gnh, gnl = _split(gn)
    pnh, pnl = _split(pn)
    ones = np.ones((1, n), dtype=BF16)

    L = np.empty((KR, n), dtype=BF16)
    L[0:3] = gh.T
    L[3:6] = gh.T
    L[6:9] = gl.T
    L[9] = ones
    L[10] = ones
    L[11] = gnh
    L[12] = gnl

    R = np.empty((KR, n), dtype=BF16)
    R[0:3] = ph.T
    R[3:6] = pl.T
    R[6:9] = ph.T
    R[9] = pnh
    R[10] = pnl
    R[11] = ones
    R[12] = ones
    return np.ascontiguousarray(L), np.ascontiguousarray(R)


def shard_inputs(preds, gts, bloc=BLOC, ncores=NCORES):
    preds = np.asarray(preds, dtype=np.float32)
    gts = np.asarray(gts, dtype=np.float32)
    n = preds.shape[1]
    ident = np.eye(128, dtype=BF16)
    in_maps = []
    for c in range(ncores):
        Ls = np.empty((bloc, KR, n), dtype=BF16)
        Rs = np.empty((bloc, KR, n), dtype=BF16)
        for b in range(bloc):
            Ls[b], Rs[b] = make_blocks(gts[c * bloc + b], preds[c * bloc + b])
        in_maps.append({"Lblk": Ls, "Rblk": Rs, "ident": ident})
    return in_maps


def combine_outputs(outs, n=N_FULL, b=B):
    tot = np.sum([o.astype(np.float64).sum() for o in outs])
    return np.float32(-2.0 * tot / (b * n))


def kernel(preds, gts):
    nc, _, _, _, on = build()
    in_maps = shard_inputs(preds, gts)
    res = run_bass_kernel_spmd(nc, in_maps, core_ids=list(range(NCORES)))
    return combine_outputs([r[on] for r in res.results])


def _numpy_chamfer(preds, gts):
    tot = 0.0
    for b_ in range(preds.shape[0]):
        gg = (gts[b_] ** 2).sum(-1)
        pp = (preds[b_] ** 2).sum(-1)
        zz = gts[b_] @ preds[b_].T
        P = gg[:, None] + pp[None, :] - 2 * zz
        tot += P.min(axis=0).mean() + P.min(axis=1).mean()
    return tot / preds.shape[0]


if __name__ == "__main__":
    from concourse.bass_interp import CoreSim

    n = int(sys.argv[1]) if len(sys.argv) > 1 else 512
    bloc = int(sys.argv[2]) if len(sys.argv) > 2 else 1
    reps = int(sys.argv[3]) if len(sys.argv) > 3 else 1
    nc, ln, rn, idn, on = build(n=n, bloc=bloc, reps=reps)
    rng = np.random.default_rng(0)
    preds = rng.standard_normal((bloc, n, D)).astype(np.float32)
    gts = rng.standard_normal((bloc, n, D)).astype(np.float32)

    sim = CoreSim(nc)
    for bi in range(bloc):
        Lb, Rb = make_blocks(gts[bi], preds[bi])
        sim.tensor(ln)[bi] = Lb
        sim.tensor(rn)[bi] = Rb
    sim.tensor(idn)[:] = np.eye(128, dtype=BF16)
    sim.simulate()
    got = combine_outputs([sim.tensor(on)], n=n, b=bloc)
    want = _numpy_chamfer(preds, gts)
    print("sim:", got, "numpy:", want, "rel err:", abs(got - want) / abs(want))

